# revision 1
# baseline (speedup 1.0000x reference)
"""Biased multi-head attention on 8 Trainium2 NeuronCores.

Strategy (head-sharded tensor parallelism):
  - 16 heads / 8 cores -> 2 heads per core. Every core runs the SAME program
    on different weight slices (Wq/Wk/Wv rows, Wo columns).
  - Host folds mask + causality into the bias (-30000 at masked entries so
    exp underflows to exactly 0), compacts away fully-masked key columns,
    and skips upper-triangle score tiles entirely.
  - The bias is injected into PSUM with an identity matmul; the score matmuls
    accumulate on top, so ACT computes exp(qk + b) straight out of PSUM and
    its output feeds the attention*V matmul directly (no elementwise fixups).
  - Row sums come for free from an appended ones-column on V.
  - The kt loop is software-pipelined (scores for kt+1 land while exp(kt)
    runs) and output-projection matmuls are interleaved as PE filler.
  - Per-core partial outputs (Wo column slice) are summed on the host.
  - Rows whose allowed prefix is fully masked follow different reference
    semantics; the host recomputes those few rows exactly.
"""

import os
import sys
from collections import deque
from contextlib import ExitStack

import numpy as np

sys.path.insert(0, "/opt/trn_rl_repo")

import ml_dtypes

S = 4096
D = 1024
H = 16
DK = 64
DV = 64
NEG = -1000000000.0
MASKNEG = -30000.0
NCORES = 8
QC = 512  # q-chunk (one PSUM bank of fp32)

BF16 = ml_dtypes.bfloat16

LAST_RESULT = None  # BassKernelResults of the most recent run (for test.py)


def _build_nc(cfg):
    """Build the (single) Bass program all 8 cores run.

    cfg: S, D, Kp (padded compacted key count), kts (kt counts per q-chunk),
    qc (q chunk size), stage (truncation for bisection).
    """
    import concourse.bass as bass
    import concourse.tile as tile
    from concourse import bacc, mybir

    dt = mybir.dt
    stage = cfg.get("stage", 5)
    S_, D_, Kp, kts, qc = cfg["S"], cfg["D"], cfg["Kp"], cfg["kts"], cfg["qc"]
    NQ = S_ // qc
    DCH = D_ // 128
    KT = Kp // 128
    assert len(kts) == NQ

    nc = bacc.Bacc(
        "TRN2",
        target_bir_lowering=False,
        debug=False,
        enable_asserts=False,
        num_devices=NCORES,
    )

    xT_d = nc.dram_tensor("xT", (D_, S_), dt.bfloat16, kind="ExternalInput").ap()
    xkvT_d = nc.dram_tensor("xkvT", (D_, Kp), dt.bfloat16, kind="ExternalInput").ap()
    BT_d = nc.dram_tensor("BT", (Kp, S_), dt.bfloat16, kind="ExternalInput").ap()
    wq_d = nc.dram_tensor("wqT", (D_, 128), dt.bfloat16, kind="ExternalInput").ap()
    wk_d = nc.dram_tensor("wkT", (D_, 128), dt.bfloat16, kind="ExternalInput").ap()
    wv_d = nc.dram_tensor("wvT", (D_, 128), dt.bfloat16, kind="ExternalInput").ap()
    wo_d = nc.dram_tensor("woT", (128, D_), dt.bfloat16, kind="ExternalInput").ap()
    id_d = nc.dram_tensor("id128", (128, 128), dt.bfloat16, kind="ExternalInput").ap()
    yT_d = nc.dram_tensor("yT", (D_, S_), dt.float32, kind="ExternalOutput").ap()

    f32 = dt.float32
    f32r = dt.float32r
    bf = dt.bfloat16
    EXP = mybir.ActivationFunctionType.Exp

    with tile.TileContext(nc) as tc, ExitStack() as ctx:
        const = ctx.enter_context(tc.tile_pool(name="const", bufs=1))
        btpool = ctx.enter_context(tc.tile_pool(name="btpool", bufs=6))
        pepool = ctx.enter_context(tc.tile_pool(name="pepool", bufs=3))
        snpool = ctx.enter_context(tc.tile_pool(name="snpool", bufs=2))
        yepool = ctx.enter_context(tc.tile_pool(name="yepool", bufs=3))
        smpool = ctx.enter_context(tc.tile_pool(name="smpool", bufs=2))
        st_ps = ctx.enter_context(tc.tile_pool(name="st_ps", bufs=2, space="PSUM"))
        av_ps = ctx.enter_context(tc.tile_pool(name="av_ps", bufs=2, space="PSUM"))
        mm_ps = ctx.enter_context(tc.tile_pool(name="mm_ps", bufs=2, space="PSUM"))

        # ---- load inputs (weights first; inputs spread over issue queues) ----
        wq_sb = const.tile([128, DCH, 128], bf, tag="wq")
        nc.scalar.dma_start(wq_sb[:, :, :], wq_d.rearrange("(c p) m -> p c m", p=128))
        wk_sb = const.tile([128, DCH, 128], bf, tag="wk")
        nc.scalar.dma_start(wk_sb[:, :, :], wk_d.rearrange("(c p) m -> p c m", p=128))
        wv_sb = const.tile([128, DCH, 128], bf, tag="wv")
        nc.scalar.dma_start(wv_sb[:, :, :], wv_d.rearrange("(c p) m -> p c m", p=128))
        wo_sb = const.tile([128, D_], bf, tag="wo")
        nc.scalar.dma_start(wo_sb[:, :], wo_d[:, :])
        id_sb = const.tile([128, 128], bf, tag="id")
        nc.scalar.dma_start(id_sb[:, :], id_d[:, :])

        xT_sb = const.tile([128, DCH, S_], bf, tag="xT")
        # first q-chunk arrives fast (small DMAs), the rest in halves
        xcuts = [0, qc] + [
            c for c in (S_ // 2, S_) if c > qc
        ]
        for a, b in zip(xcuts, xcuts[1:]):
            qs = slice(a, b)
            for dc in range(DCH):
                nc.sync.dma_start(
                    xT_sb[:, dc, qs], xT_d[dc * 128 : (dc + 1) * 128, qs]
                )
        xkvT_sb = const.tile([128, DCH, Kp], bf, tag="xkvT")
        kh = ((Kp // 2) // 128) * 128
        for a, b in ((0, kh), (kh, Kp)):
            for dc in range(DCH):
                nc.gpsimd.dma_start(
                    xkvT_sb[:, dc, a:b], xkvT_d[dc * 128 : (dc + 1) * 128, a:b]
                )

        # ones row at partition 0 (rank-1 reciprocal broadcast)
        ones_sb = const.tile([128, 64], f32, tag="ones")
        nc.vector.memset(ones_sb[0:1, :], 1.0)

        # ---- projections ----
        # qT rows 0:64 = head1 (pre-scaled by 1/sqrt(DK)), 64:128 = head2.
        # Only the pieces needed by q-chunk 0 are emitted upfront; the rest
        # are queued as PE filler inside the main loop.
        qT_sb = const.tile([128, S_], bf, tag="qT")
        kT_sb = const.tile([128, Kp], bf, tag="kT")
        vT_sb = const.tile([128, Kp], bf, tag="vT")
        v1_sb = const.tile([128, KT, 65], bf, tag="v1")
        v2_sb = const.tile([128, KT, 65], bf, tag="v2")
        nc.vector.memset(v1_sb[:, :, 64:65], 1.0)
        nc.vector.memset(v2_sb[:, :, 64:65], 1.0)

        kchunks = []
        a = 0
        while a < Kp:
            b = min(a + qc, Kp)
            kchunks.append((a, b))
            a = b

        def emit_qp(j):
            qs = slice(j * qc, (j + 1) * qc)
            ps = mm_ps.tile([128, qc], f32, tag="mm")
            for dc in range(DCH):
                nc.tensor.matmul(
                    ps[:, :],
                    lhsT=wq_sb[:, dc, :],
                    rhs=xT_sb[:, dc, qs],
                    start=(dc == 0),
                    stop=(dc == DCH - 1),
                )
            nc.vector.tensor_copy(qT_sb[:, qs], ps[:, :])

        def emit_kp(ci):
            a, b = kchunks[ci]
            ps = mm_ps.tile([128, qc], f32, tag="mm")
            for dc in range(DCH):
                nc.tensor.matmul(
                    ps[:, 0 : b - a],
                    lhsT=wk_sb[:, dc, :],
                    rhs=xkvT_sb[:, dc, a:b],
                    start=(dc == 0),
                    stop=(dc == DCH - 1),
                )
            nc.vector.tensor_copy(kT_sb[:, a:b], ps[:, 0 : b - a])

        def emit_vt(ci):
            a, b = kchunks[ci]
            ps = mm_ps.tile([128, qc], f32, tag="mm")
            for dc in range(DCH):
                nc.tensor.matmul(
                    ps[:, 0 : b - a],
                    lhsT=wv_sb[:, dc, :],
                    rhs=xkvT_sb[:, dc, a:b],
                    start=(dc == 0),
                    stop=(dc == DCH - 1),
                )
            nc.vector.tensor_copy(vT_sb[:, a:b], ps[:, 0 : b - a])
            for kt in range(a // 128, b // 128):
                ksl = slice(kt * 128, (kt + 1) * 128)
                tr = mm_ps.tile([128, 128], bf, tag="mm")
                nc.tensor.transpose(tr[:, :], vT_sb[:, ksl], id_sb[:, :])
                nc.vector.tensor_copy(v1_sb[:, kt, 0:64], tr[:, 0:64])
                nc.vector.tensor_copy(v2_sb[:, kt, 0:64], tr[:, 64:128])

        for j in range(NQ):
            emit_qp(j)
        for ci in range(len(kchunks)):
            emit_kp(ci)
        for ci in range(len(kchunks)):
            emit_vt(ci)

        # ---- attention main loop (software-pipelined over kt) ----
        rw = max(1, qc // 128)  # reshaped reciprocal width
        fillers = deque()  # pending output-projection emitters (PE filler)

        def emit_st(j, kt):
            """bias inject + score matmuls + exp for (q-chunk j, k-tile kt)."""
            qs = slice(j * qc, (j + 1) * qc)
            ksl = slice(kt * 128, (kt + 1) * 128)
            bt = btpool.tile([128, qc], bf, tag="bt")
            nc.gpsimd.dma_start(bt[:, :], BT_d[ksl, qs])
            st = st_ps.tile([128, 2 * qc], f32, tag="st")
            nc.tensor.matmul(
                st[:, 0:qc], lhsT=id_sb[:, :], rhs=bt[:, :], start=True, stop=False
            )
            nc.tensor.matmul(
                st[:, qc : 2 * qc],
                lhsT=id_sb[:, :],
                rhs=bt[:, :],
                start=True,
                stop=False,
            )
            nc.tensor.matmul(
                st[:, 0:qc],
                lhsT=kT_sb[0:64, ksl],
                rhs=qT_sb[0:64, qs],
                start=False,
                stop=True,
            )
            nc.tensor.matmul(
                st[:, qc : 2 * qc],
                lhsT=kT_sb[64:128, ksl],
                rhs=qT_sb[64:128, qs],
                start=False,
                stop=True,
            )
            pe = pepool.tile([128, 2 * qc], bf, tag="pe")
            nc.scalar.activation(pe[:, :], st[:, :], EXP)
            return pe

        def make_oproj(j, sn):
            qs = slice(j * qc, (j + 1) * qc)

            def emit(dti):
                dsl = slice(dti * 128, (dti + 1) * 128)
                yp = mm_ps.tile([128, qc], f32, tag="mm")
                nc.tensor.matmul(
                    yp[:, :], lhsT=wo_sb[:, dsl], rhs=sn[:, :], start=True, stop=True
                )
                ye = yepool.tile([128, qc], f32, tag="ye")
                if dti % 2 == 0:
                    nc.vector.tensor_copy(ye[:, :], yp[:, :])
                else:
                    nc.scalar.copy(ye[:, :], yp[:, :])
                nc.sync.dma_start(yT_d[dsl, qs], ye[:, :])

            return [lambda dti=dti: emit(dti) for dti in range(DCH)]

        for j in range(NQ if stage >= 2 else 0):
            qs = slice(j * qc, (j + 1) * qc)
            nkt = kts[j]
            sn = snpool.tile([128, qc], bf, tag="sn")
            if nkt == 0:
                nc.vector.memset(sn[:, :], 0.0)
            else:
                av1 = av_ps.tile([65, qc], f32, tag="av")
                av2 = av_ps.tile([65, qc], f32, tag="av")
                pe_next = emit_st(j, 0)
                for kt in range(nkt):
                    pe = pe_next
                    if kt + 1 < nkt:
                        pe_next = emit_st(j, kt + 1)
                    if fillers:
                        fillers.popleft()()
                    if stage < 3:
                        continue
                    nc.tensor.matmul(
                        av1[:, :],
                        lhsT=v1_sb[:, kt, :],
                        rhs=pe[:, 0:qc],
                        start=(kt == 0),
                        stop=(kt == nkt - 1),
                    )
                    nc.tensor.matmul(
                        av2[:, :],
                        lhsT=v2_sb[:, kt, :],
                        rhs=pe[:, qc : 2 * qc],
                        start=(kt == 0),
                        stop=(kt == nkt - 1),
                    )
                # normalize: sn[0:64] = av1[0:64] / rowsum1; evacuate PSUM fast,
                # reciprocal runs wide on a DMA-reshaped [128, rw] layout.
                for h, av in ((0, av1), (1, av2)) if stage >= 4 else ():
                    avs = smpool.tile([128, qc], f32, tag="avs")
                    nc.vector.tensor_copy(avs[0:65, :], av[0:65, :])
                    rsm = smpool.tile([128, 2 * rw], f32, tag="rsm")
                    nc.sync.dma_start(rsm[:, 0:rw], avs[64:65, :])
                    nc.vector.reciprocal(rsm[:, rw : 2 * rw], rsm[:, 0:rw])
                    rr = smpool.tile([1, qc], f32, tag="rr")
                    nc.sync.dma_start(rr[0:1, :], rsm[:, rw : 2 * rw])
                    recb = mm_ps.tile([64, qc], f32, tag="mm")
                    nc.tensor.matmul(
                        recb[:, :],
                        lhsT=ones_sb[0:1, :].bitcast(f32r),
                        rhs=rr[0:1, :].bitcast(f32r),
                        start=True,
                        stop=True,
                    )
                    rb = smpool.tile([64, qc], f32, tag="rb")
                    nc.vector.tensor_copy(rb[:, :], recb[:, :])
                    if h == 0:
                        nc.vector.tensor_mul(sn[0:64, :], avs[0:64, :], rb[:, :])
                    else:
                        sn2t = smpool.tile([64, qc], bf, tag="sn2t")
                        nc.vector.tensor_mul(sn2t[:, :], avs[0:64, :], rb[:, :])
                        nc.gpsimd.dma_start(sn[64:128, :], sn2t[:, :])

            if stage >= 5:
                while fillers:  # drain any leftovers before queuing chunk j
                    fillers.popleft()()
                fillers.extend(make_oproj(j, sn))

        while fillers:
            fillers.popleft()()

    return nc


def _prep_host(x, spatial_bias, mask):
    """Shared (core-independent) host preprocessing."""
    mask = np.asarray(mask).astype(bool)
    x = np.asarray(x, dtype=np.float32)
    bias = np.asarray(spatial_bias, dtype=np.float32)
    S_ = x.shape[0]
    D_ = x.shape[1]

    keep = np.flatnonzero(~mask)
    nk = int(len(keep))
    Kp = max(128, ((nk + 127) // 128) * 128)

    xT = np.ascontiguousarray(x.T).astype(BF16)
    xkvT = np.zeros((D_, Kp), dtype=BF16)
    if nk:
        xkvT[:, :nk] = x[keep].T.astype(BF16)

    # B^T [Kp, S]: bias[q, keep[j]] for keep[j] <= q else MASKNEG
    BT = np.full((Kp, S_), np.float32(MASKNEG), dtype=np.float32)
    if nk:
        b = bias.T[keep]  # [nk, S] : b[j, q] = bias[q, keep[j]]
        causal = keep[:, None] <= np.arange(S_)[None, :]
        BT[:nk] = np.where(causal, b, np.float32(MASKNEG))
    BT = BT.astype(BF16)

    # per q-chunk: number of 128-wide k tiles that contain any allowed column
    NQ = S_ // QC
    kts = []
    for j in range(NQ):
        hi = (j + 1) * QC
        cnt = int(np.searchsorted(keep, hi))
        kts.append((cnt + 127) // 128)
    return mask, keep, Kp, xT, xkvT, BT, kts


def _fixup_rows(y, x, bias, mask, Wq, Wk, Wv, Wo):
    """Exact fp32 recompute of the degenerate prefix rows (all allowed
    columns masked -> reference attends uniformly over -1e9 entries)."""
    S_, D_ = x.shape
    rows = []
    for q in range(S_):
        if not mask[q]:
            break
        rows.append(q)
    if not rows:
        return y
    H_ = Wq.shape[0] // DK
    q_p = (x @ Wq.T).reshape(S_, H_, DK).transpose(1, 0, 2)[:, rows]
    k_p = (x @ Wk.T).reshape(S_, H_, DK).transpose(1, 0, 2)
    v_p = (x @ Wv.T).reshape(S_, H_, DV).transpose(1, 0, 2)
    scores = np.einsum("hqd,hkd->hqk", q_p, k_p).astype(np.float32) / np.sqrt(
        np.float32(DK)
    )
    scores = (scores + bias[None, rows, :]).astype(np.float32)
    scores = np.where(mask[None, None, :], np.float32(NEG), scores)
    causal = np.triu(np.full((S_, S_), np.float32(NEG), dtype=np.float32), k=1)[rows]
    scores = (scores + causal[None, :, :]).astype(np.float32)
    m = scores.max(axis=-1, keepdims=True)
    e = np.exp(scores - m, dtype=np.float32)
    attn = e / e.sum(axis=-1, keepdims=True)
    out = np.einsum("hqk,hkd->hqd", attn.astype(np.float32), v_p)
    out = out.transpose(1, 0, 2).reshape(len(rows), H_ * DV)
    y[rows] = (out @ Wo.T).astype(np.float32)
    return y


def kernel(x, spatial_bias, mask, Wq, Wk, Wv, Wo):
    global LAST_RESULT
    from concourse import bass_utils

    x = np.asarray(x, dtype=np.float32)
    bias = np.asarray(spatial_bias, dtype=np.float32)
    Wq = np.asarray(Wq, dtype=np.float32)
    Wk = np.asarray(Wk, dtype=np.float32)
    Wv = np.asarray(Wv, dtype=np.float32)
    Wo = np.asarray(Wo, dtype=np.float32)
    S_, D_ = x.shape

    mask_b, keep, Kp, xT, xkvT, BT, kts = _prep_host(x, bias, mask)

    cfg = {"S": S_, "D": D_, "Kp": Kp, "kts": tuple(kts), "qc": QC}
    nc = _build_nc(cfg)
    nc.compile()

    scale = 1.0 / np.sqrt(np.float32(DK))
    id128 = np.eye(128, dtype=np.float32).astype(BF16)
    in_maps = []
    for c in range(NCORES):
        r = slice(128 * c, 128 * (c + 1))
        in_maps.append(
            {
                "xT": xT,
                "xkvT": xkvT,
                "BT": BT,
                "wqT": np.ascontiguousarray((Wq[r] * scale).T).astype(BF16),
                "wkT": np.ascontiguousarray(Wk[r].T).astype(BF16),
                "wvT": np.ascontiguousarray(Wv[r].T).astype(BF16),
                "woT": np.ascontiguousarray(Wo[:, r].T).astype(BF16),
                "id128": id128,
            }
        )

    res = bass_utils.run_bass_kernel_spmd(
        nc, in_maps, core_ids=list(range(NCORES))
    )
    LAST_RESULT = res

    yT = np.zeros((D_, S_), dtype=np.float64)
    for c in range(NCORES):
        yT += res.results[c]["yT"].astype(np.float64)
    y = np.ascontiguousarray(yT.T).astype(np.float32)

    y = _fixup_rows(y, x, bias, mask_b, Wq, Wk, Wv, Wo)
    return y

